# revision 2
# baseline (speedup 1.0000x reference)
"""KV page-cache scatter update on 8 Trainium2 NeuronCores.

Strategy (paged-attention style): shard kv_pages along the page axis —
128 pages per core.  On the host, route each valid token to the core
owning its destination page (cheap int math on 8192 indices).  Each core
then:
  1. bulk-copies its kv_pages shard to the output shard (HWDGE DMA,
     DRAM->DRAM, in chunks),
  2. concurrently gathers the routed tokens' K and V rows from HBM into
     SBUF with indirect DMA (one slot = 16*128 f32 = 8KB contiguous; K is
     the first 4KB, V the second),
  3. scatters the combined 8KB rows into the output shard with indirect
     DMA once the bulk copy of the covering region has landed.

Tokens are sorted by destination slot so each scatter group covers a
contiguous slot range and can start as soon as the copy chunks covering
that range are done (copy chunks complete in issue order on the HWDGE
FIFO).  Padding entries point at a dropped-by-bounds-check slot.
"""

import numpy as np

import concourse.bass as bass
import concourse.mybir as mybir
from concourse.bass import IndirectOffsetOnAxis
from concourse.bass_utils import run_bass_kernel_spmd

NUM_PAGES = 1024
PAGE_SIZE = 64
KV_HEADS = 8
HEAD_DIM = 128
NUM_TOKENS = 8192

N_CORES = 8
PAGES_PER_CORE = NUM_PAGES // N_CORES          # 128
SLOTS = PAGES_PER_CORE * PAGE_SIZE             # 8192 slots per core
ROW = 2 * KV_HEADS * HEAD_DIM                  # 2048 f32 per slot (8KB)
HALF = KV_HEADS * HEAD_DIM                     # 1024 f32 (4KB)
GRP = 128                                      # tokens per scatter group

# Pad sentinel: one past the last valid slot — fails the bounds check so the
# scatter drops it, and idx*row_stride stays far below int32 overflow.
DROP = np.int32(SLOTS)

LAST_RESULTS = None  # set by kernel(); lets test.py read exec_time_ns


def build_nc(n_grp: int, n_chunk: int, slots: int = SLOTS, row: int = ROW,
             half: int = HALF, num_tokens: int = NUM_TOKENS, grp: int = GRP):
    """Build the per-core SPMD Bass program.

    Inputs (per core): kv [slots,row] shard, kn/vn [num_tokens,half] full,
    ti/di [grp,n_grp] i32 (token ids / dest slots, one group per column,
    chunk-sorted by dest).  Output: out [slots,row].
    """
    f32 = mybir.dt.float32
    i32 = mybir.dt.int32
    nc = bass.Bass()
    kv = nc.declare_dram_parameter("kv", [slots, row], f32, isOutput=False)
    kn = nc.declare_dram_parameter("kn", [num_tokens, half], f32, isOutput=False)
    vn = nc.declare_dram_parameter("vn", [num_tokens, half], f32, isOutput=False)
    ti = nc.declare_dram_parameter("ti", [grp, n_grp], i32, isOutput=False)
    di = nc.declare_dram_parameter("di", [grp, n_grp], i32, isOutput=False)
    out = nc.declare_dram_parameter("out", [slots, row], f32, isOutput=True)

    chunk_rows = slots // n_chunk

    with (
        nc.sbuf_tensor([grp, n_grp * row], f32) as kvt,
        nc.sbuf_tensor([grp, n_grp], i32) as ti_sb,
        nc.sbuf_tensor([grp, n_grp], i32) as di_sb,
        nc.semaphore("copy_sem") as copy_sem,
        nc.semaphore("idx_sem") as idx_sem,
        nc.semaphore("gat_sem") as gat_sem,
        nc.semaphore("scat_sem") as scat_sem,
        nc.Block() as block,
    ):

        @block.sync
        def _(sync):
            for i in range(n_chunk):
                r = slice(i * chunk_rows, (i + 1) * chunk_rows)
                sync.dma_start(out=out[r, :], in_=kv[r, :]).then_inc(copy_sem, 16)

        @block.gpsimd
        def _(g):
            g.dma_start(out=ti_sb[:, :], in_=ti[:, :]).then_inc(idx_sem, 16)
            g.dma_start(out=di_sb[:, :], in_=di[:, :]).then_inc(idx_sem, 16)
            g.wait_ge(idx_sem, 32)
            for j in range(n_grp):
                g.indirect_dma_start(
                    out=kvt[:, j * row : j * row + half],
                    out_offset=None,
                    in_=kn[:, :],
                    in_offset=IndirectOffsetOnAxis(ap=ti_sb[:, j : j + 1], axis=0),
                ).then_inc(gat_sem, 16)
                g.indirect_dma_start(
                    out=kvt[:, j * row + half : (j + 1) * row],
                    out_offset=None,
                    in_=vn[:, :],
                    in_offset=IndirectOffsetOnAxis(ap=ti_sb[:, j : j + 1], axis=0),
                ).then_inc(gat_sem, 16)
            g.wait_ge(copy_sem, n_chunk * 16)
            g.wait_ge(gat_sem, n_grp * 32)
            for j in range(n_grp):
                g.indirect_dma_start(
                    out=out[:, :],
                    out_offset=IndirectOffsetOnAxis(ap=di_sb[:, j : j + 1], axis=0),
                    in_=kvt[:, j * row : (j + 1) * row],
                    in_offset=None,
                    bounds_check=slots - 1,
                    oob_is_err=False,
                ).then_inc(scat_sem, 16)
            g.wait_ge(scat_sem, n_grp * 16)

    return nc


_cache = {}


def _get_nc(n_grp: int, n_chunk: int):
    key = (n_grp, n_chunk)
    if key not in _cache:
        _cache[key] = build_nc(n_grp, n_chunk)
    return _cache[key]


def _route(token_dests: np.ndarray):
    """Host-side routing: per core, chunk-sorted (token_id, local_slot)
    arrays padded to a multiple of GRP.  Returns (ti, di, n_grp) with
    ti/di of shape [N_CORES, GRP, n_grp] (group g in column g)."""
    dests = token_dests.astype(np.int64)
    valid = np.nonzero(dests >= 0)[0]
    d = dests[valid]
    core = d // SLOTS
    local = d - core * SLOTS

    per_tok, per_loc = [], []
    max_n = 1
    for c in range(N_CORES):
        sel = np.nonzero(core == c)[0]
        order = np.argsort(local[sel], kind="stable")
        sel = sel[order]
        per_tok.append(valid[sel].astype(np.int32))
        per_loc.append(local[sel].astype(np.int32))
        max_n = max(max_n, len(sel))

    n_grp = -(-max_n // GRP)
    cap = n_grp * GRP
    ti = np.zeros((N_CORES, cap), np.int32)
    di = np.full((N_CORES, cap), DROP, np.int32)
    for c in range(N_CORES):
        n = len(per_tok[c])
        ti[c, :n] = per_tok[c]
        di[c, :n] = per_loc[c]
    # [cap] -> [n_grp, GRP] -> transpose to [GRP, n_grp] so group g's 128
    # indices live in column g (one offset per SBUF partition).
    ti = np.ascontiguousarray(ti.reshape(N_CORES, n_grp, GRP).transpose(0, 2, 1))
    di = np.ascontiguousarray(di.reshape(N_CORES, n_grp, GRP).transpose(0, 2, 1))
    return ti, di, n_grp


def kernel(kv_pages: np.ndarray, new_k: np.ndarray, new_v: np.ndarray,
           token_dests: np.ndarray) -> np.ndarray:
    global LAST_RESULTS
    kv_pages = np.ascontiguousarray(np.asarray(kv_pages, np.float32))
    kn = np.ascontiguousarray(np.asarray(new_k, np.float32)).reshape(NUM_TOKENS, HALF)
    vn = np.ascontiguousarray(np.asarray(new_v, np.float32)).reshape(NUM_TOKENS, HALF)
    token_dests = np.asarray(token_dests)

    ti, di, n_grp = _route(token_dests)
    n_chunk = 8
    nc = _get_nc(n_grp, n_chunk)

    kv_flat = kv_pages.reshape(N_CORES, SLOTS, ROW)
    in_maps = [
        {"kv": kv_flat[c], "kn": kn, "vn": vn, "ti": ti[c], "di": di[c]}
        for c in range(N_CORES)
    ]
    res = run_bass_kernel_spmd(nc, in_maps, list(range(N_CORES)))
    LAST_RESULTS = res
    out = np.concatenate([res.results[c]["out"][None] for c in range(N_CORES)], axis=0)
    return out.reshape(NUM_PAGES, PAGE_SIZE, 2 * KV_HEADS, HEAD_DIM)
